# revision 20
# baseline (speedup 1.0000x reference)
"""AnomalyAttention (two causal attentions per (b,h)) on 8 TRN2 NeuronCores.

Sharding: B*H = 16 (batch, head) pairs -> 2 pairs per core. Each core runs
4 independent causal attentions (time + channel for each of its 2 pairs).
No cross-core communication.

Per-attention layout ("transposed flash"): keys live on SBUF partitions.
  S^T[k, q]   = kT_tile.T @ qT                (PE bf16, contraction E=64;
                the two attn types use PE row-groups 0-63 / 64-127)
  P^T         = exp(scale * S^T)              (ACT, PSUM -> SBUF bf16)
  diag mask   = affine_select zero-triangle   (GPSIMD, on the diag block)
  outT[d, q] += V_ext[k, d].T @ P^T[k, q]     (PE bf16, PSUM-accumulated)
V_ext carries a ones column so row 64 of outT accumulates the softmax
denominator.

Queries are processed in 512-wide passes so every PSUM tile is one bank
and both score and PV psum pools double-buffer (score k+1 never waits on
exp k draining the bank). The whole (g, pass, keytile) iteration is one
flat software-pipelined loop; PV matmuls trail the score matmuls by one
key tile. Epilogue per (attn, pass): fast-reciprocal of the denominator
row [1,512] on DVE, partition-broadcast via GPSIMD copy, single DVE
multiply reading PSUM directly, bf16 DMA out. Input DMAs are staged in
first-use order so the PE starts after ~0.4 MB instead of 3 MB.
"""

import math
from contextlib import ExitStack

import ml_dtypes
import numpy as np

import concourse.bacc as bacc
import concourse.mybir as mybir
import concourse.tile as tile
from concourse.bass_utils import run_bass_kernel_spmd

B, L, H, E, D = 2, 2048, 8, 64, 64
NCORES = 8
PAIRS = (B * H) // NCORES          # (b,h) pairs per core = 2
NATT = 2 * PAIRS                   # attentions per core = 4
SCALE = 1.0 / math.sqrt(E)
P = 128                            # partitions / key-tile size
NKT = L // P                       # 16 key tiles
Q = 512                            # query pass width (one PSUM bank)
NQP = L // Q                       # 4 query passes
F32 = mybir.dt.float32
BF16 = mybir.dt.bfloat16

_CACHE = {}


def _build_nc():
    nc = bacc.Bacc()
    qt = nc.declare_dram_parameter("qt", [P, PAIRS, L], BF16, isOutput=False)
    kt = nc.declare_dram_parameter("kt", [P, PAIRS, L], BF16, isOutput=False)
    ve = nc.declare_dram_parameter("ve", [P, NATT, NKT, D + 1], BF16, isOutput=False)
    out = nc.declare_dram_parameter("out", [NATT, NQP, D, Q], BF16, isOutput=True)

    with tile.TileContext(nc) as tc:
        with ExitStack() as ctx:
            _body(ctx, tc, qt, kt, ve, out)
    nc.finalize()
    return nc


def _body(ctx, tc, qt, kt, ve, out):
    nc = tc.nc
    Exp = mybir.ActivationFunctionType.Exp

    persist = ctx.enter_context(tc.tile_pool(name="persist", bufs=1))
    # 8 PSUM banks: scores 2 tags x 3 bufs (one bank each) + pv 2 tags x 1
    s_psum = ctx.enter_context(tc.tile_pool(name="s_psum", bufs=3, space="PSUM"))
    pv_psum = ctx.enter_context(tc.tile_pool(name="pv_psum", bufs=1, space="PSUM"))
    p_pool = ctx.enter_context(tc.tile_pool(name="p_pool", bufs=7))
    o_pool = ctx.enter_context(tc.tile_pool(name="o_pool", bufs=3))
    small = ctx.enter_context(tc.tile_pool(name="small", bufs=3))

    qt_sb = persist.tile([P, PAIRS, L], BF16)
    kt_sb = persist.tile([P, PAIRS, L], BF16)
    ve_sb = persist.tile([P, NATT, NKT, D + 1], BF16)

    # staged input loads, first-use order, round-robin over 4 DGE queues.
    # chunks keep >=2KB per-partition lines for DMA efficiency; the first
    # (critical) kt/qt pieces are split by partition halves so two queues
    # work on each
    queues = [nc.default_dma_engine, nc.scalar, nc.gpsimd, nc.sync]
    qi = 0

    def dma_in(dst, src):
        nonlocal qi
        queues[qi % len(queues)].dma_start(out=dst, in_=src)
        qi += 1

    for ph in range(2):
        p0, p1 = 64 * ph, 64 * ph + 64
        dma_in(kt_sb[p0:p1, 0, 0:Q], kt[p0:p1, 0, 0:Q])
        dma_in(qt_sb[p0:p1, 0, 0:Q], qt[p0:p1, 0, 0:Q])
    for ph in range(2):
        p0, p1 = 64 * ph, 64 * ph + 64
        dma_in(kt_sb[p0:p1, 0, Q:2 * Q], kt[p0:p1, 0, Q:2 * Q])
        dma_in(qt_sb[p0:p1, 0, Q:2 * Q], qt[p0:p1, 0, Q:2 * Q])
    for a in range(2):
        dma_in(ve_sb[:, a, :, :], ve[:, a, :, :])
    dma_in(kt_sb[:, 0, 2 * Q:L], kt[:, 0, 2 * Q:L])
    dma_in(qt_sb[:, 0, 2 * Q:L], qt[:, 0, 2 * Q:L])
    for a in range(2, 4):
        dma_in(ve_sb[:, a, :, :], ve[:, a, :, :])
    for c in range(2):
        dma_in(kt_sb[:, 1, Q * 2 * c:Q * 2 * (c + 1)], kt[:, 1, Q * 2 * c:Q * 2 * (c + 1)])
        dma_in(qt_sb[:, 1, Q * 2 * c:Q * 2 * (c + 1)], qt[:, 1, Q * 2 * c:Q * 2 * (c + 1)])

    pv_tiles = {}

    def emit_pv(g, qs, k, pTk, w, off):
        last = k == 4 * qs + 3
        for t in range(2):
            nc.tensor.matmul(
                pv_tiles[(g, qs)][t][:, off:Q],
                lhsT=ve_sb[:, 2 * g + t, k, :],
                rhs=pTk[:, t, :w],
                start=(k == 0),
                stop=last,
                skip_group_check=True,
            )
        return last

    def epilogue(g, qs):
        # in the final epilogue nothing remains to hide the serial chain
        # under, so spread work onto the otherwise-idle ACT and GPSIMD
        final = g == PAIRS - 1 and qs == NQP - 1
        Copy = mybir.ActivationFunctionType.Copy
        pvs = pv_tiles.pop((g, qs))
        # drain both PSUM tiles to SBUF first: the next pass's first PV
        # matmul WAR-waits on these banks (pv bufs=1), so releasing them
        # after one copy instead of after the whole normalize chain removes
        # the ~1.5us PE gap at every pass boundary
        obs = []
        for t in range(2):
            ob = o_pool.tile([D + 1, Q], F32, tag=f"ob{t}")
            if final and t == 1:
                nc.scalar.activation(out=ob, in_=pvs[t], func=Copy)
            else:
                nc.vector.tensor_copy(out=ob, in_=pvs[t])
            obs.append(ob)
        for t in range(2):
            a = 2 * g + t
            ob = obs[t]
            # denominator row (partition 64) -> partition 0, reciprocal,
            # replicate to partition 32, then quadrant-broadcast to 0..63
            # (cross-partition-base shifts are fine on plain DVE copies, but
            # NOT on the custom-DVE reciprocal — keep recip at matching base)
            den = small.tile([D, Q], F32, tag="den")
            rec = small.tile([D, Q], F32, tag="rec")
            if final and t == 1:
                nc.scalar.activation(out=den[0:1, :], in_=ob[D:D + 1, :], func=Copy)
            else:
                nc.vector.tensor_copy(out=den[0:1, :], in_=ob[D:D + 1, :])
            nc.vector.reciprocal_approx_fast(out=rec[0:1, :], in_=den[0:1, :])
            nc.vector.tensor_copy(out=rec[32:33, :], in_=rec[0:1, :])
            rec_b = small.tile([D, Q], F32, tag="rec_b")
            nc.vector.stream_shuffle(out=rec_b, in_=rec, mask=[0] * 32)
            o_n = o_pool.tile([D, Q], BF16, tag="o")
            if final:
                nc.gpsimd.tensor_mul(o_n, ob[0:D, :], rec_b)
            else:
                nc.vector.tensor_mul(o_n, ob[0:D, :], rec_b)
            # split the store across two queues (1KB/partition lines move at
            # ~20GB/s per queue; halving per-queue bytes halves drain time)
            qb = 2 * t
            queues[qb].dma_start(out=out[a, qs, 0:32, :], in_=o_n[0:32, :])
            queues[qb + 1].dma_start(out=out[a, qs, 32:64, :], in_=o_n[32:64, :])

    work = [(g, qs, k) for g in range(PAIRS) for qs in range(NQP)
            for k in range(4 * qs + 4)]
    # PV matmuls trail the score matmuls by DEPTH key tiles so that every
    # PE instruction's semaphore wait is satisfied well before decode (a
    # freshly-satisfied wait costs ~120ns of sequencer stall per matmul)
    DEPTH = 4
    pend = []
    for g, qs, k in work:
        q0, q1 = qs * Q, qs * Q + Q
        qlo = max(q0, P * k)
        w = q1 - qlo
        off = qlo - q0
        diag = qlo == P * k
        if k == 0:
            pv_tiles[(g, qs)] = [
                pv_psum.tile([D + 1, Q], F32, tag=f"pv{t}", name=f"pv{t}")
                for t in range(2)
            ]
        # both attn types' score matmuls target one [128, 2, Q] psum tile
        # (PE row groups 0-63 / 64-127): the t=1 matmul's psum WAR wait is
        # identical to t=0's, so it is stale at decode and the pair executes
        # concurrently on disjoint PE row tiles
        s2 = s_psum.tile([P, 2, Q], F32, tag="s", name="s")
        for t in range(2):
            bp = 64 * t
            nc.tensor.matmul(
                s2[:, t, :w],
                lhsT=kt_sb[bp:bp + 64, g, P * k:P * (k + 1)],
                rhs=qt_sb[bp:bp + 64, g, qlo:q1],
                start=True,
                stop=True,
                skip_group_check=True,
            )
        if len(pend) >= DEPTH:
            pg, pqs, pk, ppT, pw, poff = pend.pop(0)
            if emit_pv(pg, pqs, pk, ppT, pw, poff):
                epilogue(pg, pqs)
        # one exp covers both attn types ([128, 2, w] strided AP) to halve
        # the ACT per-instruction access overhead
        pT = p_pool.tile([P, 2, Q], BF16, tag="p", name="p")
        nc.scalar.activation(pT[:, :, :w], s2[:, :, :w], Exp, scale=SCALE)
        if diag:
            # diagonal block, both attn types at once: zero where q < key
            # (iota = j - part, constant across the t dim)
            nc.gpsimd.affine_select(
                out=pT[:, :, 0:P],
                in_=pT[:, :, 0:P],
                compare_op=mybir.AluOpType.is_ge,
                fill=0.0,
                base=0,
                channel_multiplier=-1,
                pattern=[[0, 2], [1, P]],
            )
        pend.append((g, qs, k, pT, w, off))
    for pg, pqs, pk, ppT, pw, poff in pend:
        if emit_pv(pg, pqs, pk, ppT, pw, poff):
            epilogue(pg, pqs)


def _host_shard(inputs):
    """Build the 8 per-core input maps from full inputs (host-side numpy)."""
    q_t = np.asarray(inputs["queries_time"], dtype=np.float32)
    k_t = np.asarray(inputs["keys_time"], dtype=np.float32)
    v_t = np.asarray(inputs["values_time"], dtype=np.float32)
    q_c = np.asarray(inputs["queries_channel"], dtype=np.float32)
    k_c = np.asarray(inputs["keys_channel"], dtype=np.float32)
    v_c = np.asarray(inputs["values_channel"], dtype=np.float32)

    bf16 = ml_dtypes.bfloat16
    in_maps = []
    for c in range(NCORES):
        qt = np.empty((P, PAIRS, L), np.float32)
        kt = np.empty((P, PAIRS, L), np.float32)
        ve = np.empty((P, NATT, NKT, D + 1), np.float32)
        for g in range(PAIRS):
            p = PAIRS * c + g
            b, h = divmod(p, H)
            qt[:64, g, :] = q_t[b, :, h, :].T
            qt[64:, g, :] = q_c[b, :, h, :].T
            kt[:64, g, :] = k_t[b, :, h, :].T
            kt[64:, g, :] = k_c[b, :, h, :].T
            for t, v_full in enumerate((v_t, v_c)):
                a = 2 * g + t
                # ve[p_row, a, ktile, 0:64] = V[ktile*128 + p_row, :]
                ve[:, a, :, :D] = v_full[b, :, h, :].reshape(NKT, P, D).transpose(1, 0, 2)
                ve[:, a, :, D] = 1.0
        in_maps.append({
            "qt": np.ascontiguousarray(qt).astype(bf16),
            "kt": np.ascontiguousarray(kt).astype(bf16),
            "ve": np.ascontiguousarray(ve).astype(bf16),
        })
    return in_maps


def _run(in_maps, trace=False):
    if "nc" not in _CACHE:
        _CACHE["nc"] = _build_nc()
    return run_bass_kernel_spmd(
        _CACHE["nc"], in_maps, core_ids=list(range(NCORES)), trace=trace
    )


def kernel(**inputs):
    in_maps = _host_shard(inputs)
    res = _run(in_maps, trace=False)
    v_time = np.empty((B, L, H, D), np.float32)
    v_chan = np.empty((B, L, H, D), np.float32)
    for c in range(NCORES):
        o = np.asarray(res.results[c]["out"]).astype(np.float32)  # [NATT,NQP,D,Q]
        for g in range(PAIRS):
            p = PAIRS * c + g
            b, h = divmod(p, H)
            v_time[b, :, h, :] = o[2 * g + 0].transpose(0, 2, 1).reshape(L, D)
            v_chan[b, :, h, :] = o[2 * g + 1].transpose(0, 2, 1).reshape(L, D)
    return v_time, v_chan
